# revision 1
# baseline (speedup 1.0000x reference)
"""Trainium2 Bass kernel for nn_EnhancedTransformerBlock_51917564674691.

Reference block (B=4, S=2048, D=256):
  x_global = global_mha(x, 8 heads, hd=32)          # dense S x S attention
  x_local  = local_mha(x, 4 heads, hd=64, window=5) # banded attention
  x_fused  = MLP_512(silu) over concat([x_global, x_local])
  x        = LN(x + x_fused); x = LN(x + FFN(x)); return x

Sharding: 8 cores = 4 batches x 2 sequence-halves. Each core computes the
full-batch K/V for global attention (needs all 2048 keys) and produces the
output for its 1024 tokens.

Layouts: "T-layout" = [feature partitions, token free] throughout the matmul
chain; host pre-transposes x and all weights into SBUF-image form so every
DMA is a contiguous [128, F] block. Attention internals are bf16 (fp32 PSUM
accumulation everywhere); softmax exp runs on ScalarE from 4-bank PSUM score
tiles; denominators come from ones-matmuls (col-packed) on the PE.
"""
import os
import numpy as np
import ml_dtypes

import concourse.bass as bass
import concourse.tile as tile
from concourse import bacc, mybir
from concourse.bass_utils import run_bass_kernel_spmd
from concourse.masks import make_identity

P = 128
BF = mybir.dt.bfloat16
F32 = mybir.dt.float32
BF_NP = ml_dtypes.bfloat16

B, S, D = 4, 2048, 256
TQ = 1024           # tokens per core
XQ = 1152           # padded x_q length (own tokens + halo, zero padded)
NQT = 2             # global q tiles of 512
NKT = 16            # global key tiles of 128
GSC = 1.0 / np.sqrt(32.0)   # global attention scale
LSC = 0.125                 # local attention scale (1/sqrt(64))
LB = 124            # local block queries
NLB = 9             # local blocks (9*124 = 1116 >= 1024)
EPS = 1e-5

AF = mybir.ActivationFunctionType

# name -> (shape, np dtype) of per-core DRAM inputs (all SBUF-image [128, F])
INPUT_SPECS = {
    "xkvT": ((P, 2 * 2048), BF_NP),   # x[b].T            (full batch, T-layout)
    "xqT": ((P, 2 * XQ), BF_NP),      # x_q.T padded      (own + halo, T-layout)
    "xownN": ((P, 8 * 256), np.float32),  # x own tokens  (N-layout image)
    "wgq": ((P, 2 * 256), BF_NP),
    "wgk": ((P, 2 * 256), BF_NP),
    "wgv": ((P, 2 * 256), BF_NP),
    "wtqk": ((P, 2 * 512), BF_NP),
    "wtv": ((P, 2 * 256), BF_NP),
    "wgo": ((P, 2 * 256), BF_NP),
    "wto": ((P, 2 * 256), BF_NP),
    "wf1": ((P, 4 * 512), BF_NP),
    "wf2": ((P, 4 * 256), BF_NP),
    "wn1": ((P, 2 * 512), BF_NP),
    "wn2": ((P, 4 * 256), BF_NP),
    "bgq": ((P, 2), np.float32),
    "bgk": ((P, 2), np.float32),
    "btqk": ((P, 4), np.float32),
    "bgo": ((P, 2), np.float32),
    "bto": ((P, 2), np.float32),
    "bf1": ((P, 4), np.float32),
    "nbf1": ((P, 4), np.float32),
    "bn1": ((P, 4), np.float32),
    "nbn1": ((P, 4), np.float32),
    "bgv128": ((P, 256), np.float32),
    "btv128": ((P, 256), np.float32),
    "bf2128": ((P, 256), np.float32),
    "bn2128": ((P, 256), np.float32),
    "gng128": ((P, 256), np.float32),
    "gnb128": ((P, 256), np.float32),
    "fng128": ((P, 256), np.float32),
    "fnb128": ((P, 256), np.float32),
    "bandA": ((P, LB), BF_NP),
    "bndF": ((P, 1), BF_NP),
    "bndL": ((P, 1), BF_NP),
}


def _patch_act_tables():
    """Make Exp and Ln resolve to the combined natural_log_exp_and_others set
    so the table-load pass emits ONE load instead of thrashing between
    exp_and_others and natural_log (9 loads, ~2.7us each + ACT drains)."""
    import concourse.hw_specs as hs
    if getattr(hs, "_act_tables_patched", False):
        return
    orig = hs.get_activation_tables

    def patched(module_arch):
        t = dict(orig(module_arch))
        exp = mybir.ActivationFunctionType.Exp
        ln = mybir.ActivationFunctionType.Ln
        for name in ("exp_and_others", "exp_and_friends"):
            if name in t:
                t[name] = t[name] - {exp}
        if "natural_log" in t:
            t["natural_log"] = t["natural_log"] - {ln}
        return t

    hs.get_activation_tables = patched
    import concourse.bacc as bc
    bc.get_activation_tables = patched
    hs._act_tables_patched = True


def build():
    _patch_act_tables()
    nc = bacc.Bacc("TRN2", target_bir_lowering=False, debug=False, num_devices=8)
    dram = {}
    for name, (shape, npdt) in INPUT_SPECS.items():
        dram[name] = nc.dram_tensor(
            name, list(shape), mybir.dt.from_np(np.dtype(npdt)), kind="ExternalInput"
        ).ap()
    out_dram = nc.dram_tensor("out", [P, 8 * 256], F32, kind="ExternalOutput").ap()

    with tile.TileContext(nc) as tc:
        _emit(nc, tc, dram, out_dram)
    nc.compile()
    return nc


def _emit(nc, tc, dram, out_dram):
    from contextlib import ExitStack
    ctx = ExitStack()
    BARRIER = int(os.environ.get("KBARRIER", "0"))

    cpool = ctx.enter_context(tc.tile_pool(name="const", bufs=1))
    wpool = ctx.enter_context(tc.tile_pool(name="work", bufs=1))
    spool = ctx.enter_context(tc.tile_pool(name="scratch", bufs=4))
    epool = ctx.enter_context(tc.tile_pool(name="exps", bufs=2))
    pp = ctx.enter_context(tc.tile_pool(name="ps", bufs=1, space="PSUM"))

    TT = mybir.AluOpType

    def _kernel_body():

            # ---- load constants / inputs --------------------------------------
            cin = {}
            for name, (shape, npdt) in INPUT_SPECS.items():
                t = cpool.tile(list(shape), mybir.dt.from_np(np.dtype(npdt)), tag=name)
                nc.sync.dma_start(t[:], dram[name])
                cin[name] = t

            ones_bf = cpool.tile([P, 64], BF, tag="ones_bf")
            nc.vector.memset(ones_bf[:], 1.0)
            ident = cpool.tile([P, P], F32, tag="ident")
            make_identity(nc, ident[:])

            # reshaped views of inputs
            xkvT = cin["xkvT"][:].rearrange("p (k n) -> p k n", k=2)     # [128,2,2048] bf
            xqT = cin["xqT"][:].rearrange("p (k n) -> p k n", k=2)       # [128,2,1152]
            xownN = cin["xownN"][:].rearrange("p (t f) -> p t f", t=8)   # [128,8,256] f32
            w = {k: cin[k][:].rearrange("p (k2 n) -> p k2 n", k2=2)
                 for k in ("wgq", "wgk", "wgv", "wtqk", "wtv", "wgo", "wto", "wn1")}
            w["wf1"] = cin["wf1"][:].rearrange("p (k2 n) -> p k2 n", k2=4)
            w["wf2"] = cin["wf2"][:].rearrange("p (k2 n) -> p k2 n", k2=4)
            w["wn2"] = cin["wn2"][:].rearrange("p (k2 n) -> p k2 n", k2=4)

            # ---- persistent intermediates ------------------------------------
            qT = wpool.tile([P, 2, 1024], BF, tag="qT")
            kT = wpool.tile([P, 2, 2048], BF, tag="kT")
            v_aug = wpool.tile([P, NKT, 8, 64], BF, tag="v_aug")
            qkL = wpool.tile([P, 4, XQ], BF, tag="qkL")
            vL = wpool.tile([P, NLB, 256], BF, tag="vL")
            g_oT = wpool.tile([P, 2, 1024], BF, tag="g_oT")
            l_oT = wpool.tile([P, 2, 1024], BF, tag="l_oT")
            combT = wpool.tile([P, 4, 1024], BF, tag="combT")
            h1s = wpool.tile([P, 4, 1024], BF, tag="h1s")
            x1N = wpool.tile([P, 8, 256], F32, tag="x1N")
            x1T = wpool.tile([P, 2, 1024], BF, tag="x1T")
            h2s = wpool.tile([P, 4, 1024], BF, tag="h2s")
            out_sb = wpool.tile([P, 8, 256], F32, tag="out_sb")

            def ps_big(tag="sc"):
                return pp.tile([P, 2048], F32, tag=tag, name="ps_sc")

            def ps_av():
                return pp.tile([P, 512], F32, tag="av", bufs=2, name="ps_av")

            def ps_sm():
                return pp.tile([P, 512], F32, tag="sm", bufs=2, name="ps_sm")

            def bias_bc(name, m, n):
                return cin[name][:, m:m + 1].to_broadcast([P, n])

            # ---- qkv projections (global) ------------------------------------
            # kT = Wk @ x^T over full batch
            for m in range(2):
                for nt in range(4):
                    pm = ps_sm()
                    for k in range(2):
                        nc.tensor.matmul(pm[:], w["wgk"][:, k, 128 * m:128 * m + 128],
                                         xkvT[:, k, 512 * nt:512 * nt + 512],
                                         start=(k == 0), stop=(k == 1))
                    nc.scalar.activation(kT[:, m, 512 * nt:512 * nt + 512], pm[:],
                                         AF.Identity, bias=cin["bgk"][:, m:m + 1])
            # v (N-layout, augmented with ones columns): v[key, f] over full batch
            nc.vector.memset(v_aug[:, :, :, 32:64], 1.0)
            for mt in range(16):
                pm = ps_sm()
                for k in range(2):
                    nc.tensor.matmul(pm[:, 0:256], xkvT[:, k, 128 * mt:128 * mt + 128],
                                     w["wgv"][:, k, :], start=(k == 0), stop=(k == 1))
                nc.vector.tensor_tensor(
                    v_aug[:, mt, :, 0:32],
                    pm[:, 0:256].rearrange("p (h d) -> p h d", h=8),
                    cin["bgv128"][:].rearrange("p (h d) -> p h d", h=8), TT.add)
            # qT over own tokens (x_q rows 2..1026)
            for m in range(2):
                for nt in range(2):
                    pm = ps_sm()
                    for k in range(2):
                        nc.tensor.matmul(pm[:], w["wgq"][:, k, 128 * m:128 * m + 128],
                                         xqT[:, k, 2 + 512 * nt:2 + 512 * nt + 512],
                                         start=(k == 0), stop=(k == 1))
                    nc.scalar.activation(qT[:, m, 512 * nt:512 * nt + 512], pm[:],
                                         AF.Identity, bias=cin["bgq"][:, m:m + 1])

            # ---- qkv projections (local) -------------------------------------
            for m in range(4):
                for nt in range(3):
                    pm = ps_sm()
                    for k in range(2):
                        nc.tensor.matmul(pm[:, 0:384], w["wtqk"][:, k, 128 * m:128 * m + 128],
                                         xqT[:, k, 384 * nt:384 * nt + 384],
                                         start=(k == 0), stop=(k == 1))
                    nc.scalar.activation(qkL[:, m, 384 * nt:384 * nt + 384], pm[:, 0:384],
                                         AF.Identity, bias=cin["btqk"][:, m:m + 1])
            for blk in range(NLB):
                pm = ps_sm()
                for k in range(2):
                    nc.tensor.matmul(pm[:, 0:256], xqT[:, k, 124 * blk:124 * blk + 128],
                                     w["wtv"][:, k, :], start=(k == 0), stop=(k == 1))
                nc.vector.tensor_tensor(vL[:, blk, :], pm[:, 0:256], cin["btv128"][:], TT.add)

            PHASE = int(os.environ.get("KPHASE", "8"))
            if PHASE != 8:
                nc.vector.memset(out_sb[:], 0.0)
            if PHASE < 2:
                nc.sync.dma_start(out_dram[:, 0:2048],
                                  out_sb[:, 0:8, :].rearrange("p t f -> p (t f)"))
                ctx.close()
                return

            # ---- local attention ---------------------------------------------
            for blk in range(NLB):
                k0 = 124 * blk
                q0 = 2 + 124 * blk
                qn = 32 if blk == NLB - 1 else LB  # valid queries in this block
                psc = ps_big("sc")
                for l in range(4):
                    r = l % 2
                    pt = l // 2
                    nc.tensor.matmul(psc[:, 512 * l:512 * l + LB],
                                     qkL[64 * r:64 * r + 64, 2 + pt, k0:k0 + 128],
                                     qkL[64 * r:64 * r + 64, pt, q0:q0 + LB],
                                     start=True, stop=True, tile_position=(64 * r, 0))
                eloc = epool.tile([P, 4, LB], BF, tag="eloc")
                for l in range(4):
                    nc.scalar.activation(eloc[:, l, :], psc[:, 512 * l:512 * l + LB],
                                         AF.Exp, scale=LSC)
                nc.vector.tensor_tensor(eloc[:], eloc[:],
                                        cin["bandA"][:, None, :].to_broadcast([P, 4, LB]),
                                        TT.mult)
                if blk == 0:
                    nc.vector.tensor_tensor(eloc[:], eloc[:],
                                            cin["bndF"][:].to_broadcast([P, 4, LB]), TT.mult)
                if blk == NLB - 1:
                    nc.vector.tensor_tensor(eloc[:], eloc[:],
                                            cin["bndL"][:].to_broadcast([P, 4, LB]), TT.mult)
                pav = [ps_av(), ps_av()]
                pde = [ps_sm(), ps_sm()]
                for l in range(4):
                    pr, c = l // 2, l % 2
                    nc.tensor.matmul(pav[pr][64 * c:64 * c + 64, 0:LB],
                                     vL[:, blk, 64 * l:64 * l + 64], eloc[:, l, :],
                                     start=True, stop=True, tile_position=(0, 64 * c))
                    nc.tensor.matmul(pde[pr][64 * c:64 * c + 64, 0:LB],
                                     ones_bf[:], eloc[:, l, :],
                                     start=True, stop=True, tile_position=(0, 64 * c))
                for pr in range(2):
                    rec = spool.tile([P, LB], F32, tag="lrec")
                    nc.vector.reciprocal(rec[:], pde[pr][:, 0:LB])
                    nc.vector.tensor_tensor(l_oT[:, pr, k0:k0 + qn], pav[pr][:, 0:qn],
                                            rec[:, 0:qn], TT.mult)

            if PHASE < 3:
                nc.sync.dma_start(out_dram[:, 0:2048],
                                  out_sb[:, 0:8, :].rearrange("p t f -> p (t f)"))
                ctx.close()
                return

            # ---- per-chunk: global attention + MLP tail ----------------------
            for qt in range(NQT):
                if BARRIER:
                    tc.strict_bb_all_engine_barrier()
                for hg in range(2):
                    pav = [ps_av(), ps_av()]
                    for kt in range(NKT):
                        psc = ps_big("sc")
                        for hc in range(4):
                            nc.tensor.matmul(
                                psc[:, 512 * hc:512 * hc + 512],
                                kT[32 * hc:32 * hc + 32, hg, 128 * kt:128 * kt + 128],
                                qT[32 * hc:32 * hc + 32, hg, 512 * qt:512 * qt + 512],
                                start=True, stop=True, tile_position=(32 * hc, 0))
                        eg = epool.tile([P, 2048], BF, tag="eg")
                        nc.scalar.activation(eg[:], psc[:], AF.Exp, scale=GSC)
                        # pair p covers heads 4*hg+2p, 4*hg+2p+1:
                        #   psum rows 0:32 = o(head 2p), 32:64 = den(2p) replicated,
                        #   rows 64:96 = o(2p+1), 96:128 = den(2p+1)
                        # skip_group_check: CoreSim's zero-region tracker is
                        # partition-blind (any two concurrent groups per bank
                        # conflict); HW has per-element has_written bits and the
                        # 64-offset dual-group pattern is verified exact on HW.
                        for p2 in range(2):
                            for c in range(2):
                                h = 4 * hg + 2 * p2 + c
                                nc.tensor.matmul(pav[p2][64 * c:64 * c + 64, :],
                                                 v_aug[:, kt, h, :],
                                                 eg[:, 512 * (2 * p2 + c):512 * (2 * p2 + c) + 512],
                                                 start=(kt == 0), stop=(kt == NKT - 1),
                                                 tile_position=(0, 64 * c),
                                                 skip_group_check=True)
                    qsl = slice(512 * qt, 512 * qt + 512)
                    for p2 in range(2):
                        rec = spool.tile([P, 512], F32, tag="grec")
                        nc.vector.reciprocal(rec[0:32, :], pav[p2][32:64, :])
                        nc.vector.reciprocal(rec[64:96, :], pav[p2][96:128, :])
                        nc.vector.tensor_tensor(g_oT[64 * p2:64 * p2 + 32, hg, qsl],
                                                pav[p2][0:32, :], rec[0:32, :], TT.mult)
                        nc.vector.tensor_tensor(g_oT[64 * p2 + 32:64 * p2 + 64, hg, qsl],
                                                pav[p2][64:96, :], rec[64:96, :], TT.mult)

                if PHASE < 4:
                    continue
                if BARRIER:
                    tc.strict_bb_all_engine_barrier()
                # out projections -> combT
                if PHASE != 36:
                    for m in range(2):
                        pm = ps_sm()
                        for k in range(2):
                            nc.tensor.matmul(pm[:], w["wgo"][:, k, 128 * m:128 * m + 128],
                                             g_oT[:, k, 512 * qt:512 * qt + 512],
                                             start=(k == 0), stop=(k == 1))
                        nc.vector.tensor_tensor(combT[:, m, 512 * qt:512 * qt + 512], pm[:],
                                                bias_bc("bgo", m, 512), TT.add)
                if PHASE != 35:
                    for m in range(2):
                        pm = ps_sm()
                        for k in range(2):
                            nc.tensor.matmul(pm[:], w["wto"][:, k, 128 * m:128 * m + 128],
                                             l_oT[:, k, 512 * qt:512 * qt + 512],
                                             start=(k == 0), stop=(k == 1))
                        nc.vector.tensor_tensor(combT[:, 2 + m, 512 * qt:512 * qt + 512], pm[:],
                                                bias_bc("bto", m, 512), TT.add)

                if PHASE < 5 or PHASE in (35, 36):
                    continue
                # fused MLP gemm1 + silu (silu(x) = (x+b) / (1 + exp(-(x+b))))
                for m in range(4):
                    pm = ps_sm()
                    for k in range(4):
                        nc.tensor.matmul(pm[:], w["wf1"][:, k, 128 * m:128 * m + 128],
                                         combT[:, k, 512 * qt:512 * qt + 512],
                                         start=(k == 0), stop=(k == 3))
                    _silu(nc, spool, h1s[:, m, 512 * qt:512 * qt + 512], pm,
                          cin["bf1"], cin["nbf1"], m)

                if PHASE < 6:
                    continue
                # fused MLP gemm2 (N-layout out) + residual + LN1
                _g2_res_ln(nc, spool, pp, qt, h1s, w["wf2"], cin["bf2128"], xownN,
                           cin["gng128"], cin["gnb128"], x1N, x1N_src=None)

                if PHASE < 7:
                    continue
                # transpose x1N chunk -> x1T
                for tt in range(4):
                    ta = 4 * qt + tt
                    for fh in range(2):
                        ptr = ps_sm()
                        nc.tensor.transpose(ptr[:, 0:128], x1N[:, ta, 128 * fh:128 * fh + 128],
                                            ident[:])
                        nc.vector.tensor_copy(x1T[:, fh, 128 * ta:128 * ta + 128],
                                              ptr[:, 0:128])

                if PHASE < 8:
                    continue
                # FFN gemm1 + silu
                for m in range(4):
                    pm = ps_sm()
                    for k in range(2):
                        nc.tensor.matmul(pm[:], w["wn1"][:, k, 128 * m:128 * m + 128],
                                         x1T[:, k, 512 * qt:512 * qt + 512],
                                         start=(k == 0), stop=(k == 1))
                    _silu(nc, spool, h2s[:, m, 512 * qt:512 * qt + 512], pm,
                          cin["bn1"], cin["nbn1"], m)

                # FFN gemm2 + residual(x1N) + LN2 -> out_sb
                _g2_res_ln(nc, spool, pp, qt, h2s, w["wn2"], cin["bn2128"], None,
                           cin["fng128"], cin["fnb128"], out_sb, x1N_src=x1N)

                # store chunk
                nc.sync.dma_start(out_dram[:, 1024 * qt:1024 * qt + 1024],
                                  out_sb[:, 4 * qt:4 * qt + 4, :].rearrange("p t f -> p (t f)"))
            if PHASE in (3, 4, 5, 6, 7, 35, 36):
                nc.sync.dma_start(out_dram[:, 0:2048],
                                  out_sb[:, 0:8, :].rearrange("p t f -> p (t f)"))


    REPEAT = int(os.environ.get("KREPEAT", "1"))
    if REPEAT > 1:
        with tc.For_i(0, REPEAT, 1):
            _kernel_body()
    else:
        _kernel_body()
    ctx.close()


def _silu(nc, spool, out_ap, pm, b_t, nb_t, m):
    """out = silu(pm + b) where b is per-partition bias column m."""
    TT = mybir.AluOpType
    e = spool.tile([P, 512], F32, tag="se", name="se")
    nc.scalar.activation(e[:], pm[:], AF.Exp, bias=nb_t[:, m:m + 1], scale=-1.0)
    hb = spool.tile([P, 512], F32, tag="shb", name="shb")
    nc.vector.tensor_tensor(hb[:], pm[:], b_t[:, m:m + 1].to_broadcast([P, 512]),
                            TT.add)
    nc.vector.tensor_scalar_add(e[:], e[:], 1.0)
    nc.vector.reciprocal(e[:], e[:])
    nc.vector.tensor_tensor(out_ap, hb[:], e[:], TT.mult)


def _g2_res_ln(nc, spool, pp, qt, hsrc, w2, b128, xownN, g128, b_ln128, dest,
               x1N_src):
    """gemm2 (contract 512 -> 256, N-layout out) + bias + residual + layernorm.

    residual = xownN[:, 4qt+tt, :] if xownN is not None else x1N_src[:, 4qt+tt, :]
    dest[:, 4qt+tt, :] = LN(res + gemm2_out + b128) * g128 + b_ln128
    """
    TT = mybir.AluOpType
    xrs = []
    mu_raw = spool.tile([P, 4], F32, tag="mu_raw")
    s2_raw = spool.tile([P, 4], F32, tag="s2_raw")
    for tt in range(4):
        ta = 4 * qt + tt
        pm = pp.tile([P, 512], F32, tag="sm", bufs=2, name="ps_sm")
        for k in range(4):
            nc.tensor.matmul(pm[:, 0:256], hsrc[:, k, 128 * ta:128 * ta + 128],
                             w2[:, k, :], start=(k == 0), stop=(k == 3))
        xr = spool.tile([P, 256], F32, tag=f"xr{tt}", name="xr")
        nc.vector.tensor_tensor(xr[:], pm[:, 0:256], b128[:], TT.add)
        res = xownN[:, ta, :] if xownN is not None else x1N_src[:, ta, :]
        sq = spool.tile([P, 256], F32, tag="sq", name="sq")
        nc.vector.tensor_tensor(xr[:], xr[:], res, TT.add)
        nc.vector.reduce_sum(mu_raw[:, tt:tt + 1], xr[:], axis=mybir.AxisListType.X)
        nc.vector.tensor_tensor(sq[:], xr[:], xr[:], TT.mult)
        nc.vector.reduce_sum(s2_raw[:, tt:tt + 1], sq[:], axis=mybir.AxisListType.X)
        xrs.append(xr)
    mu = spool.tile([P, 4], F32, tag="mu")
    var = spool.tile([P, 4], F32, tag="var")
    istd = spool.tile([P, 4], F32, tag="istd")
    nc.vector.tensor_scalar_mul(mu[:], mu_raw[:], 1.0 / 256.0)
    nc.vector.tensor_scalar_mul(var[:], s2_raw[:], 1.0 / 256.0)
    nc.vector.tensor_tensor(istd[:], mu[:], mu[:], TT.mult)
    nc.vector.tensor_tensor(var[:], var[:], istd[:], TT.subtract)
    nc.vector.tensor_scalar_add(var[:], var[:], EPS)
    nc.scalar.activation(var[:], var[:], AF.Ln)
    nc.scalar.activation(istd[:], var[:], AF.Exp, scale=-0.5)
    for tt in range(4):
        ta = 4 * qt + tt
        xr = xrs[tt]
        nc.vector.tensor_tensor(xr[:], xr[:], mu[:, tt:tt + 1].to_broadcast([P, 256]),
                                TT.subtract)
        nc.vector.tensor_tensor(xr[:], xr[:], istd[:, tt:tt + 1].to_broadcast([P, 256]),
                                TT.mult)
        nc.vector.tensor_tensor(xr[:], xr[:], g128[:], TT.mult)
        nc.vector.tensor_tensor(dest[:, ta, :], xr[:], b_ln128[:], TT.add)


# ======================================================================
# Host side
# ======================================================================

_NC = None


def _get_nc():
    global _NC
    if _NC is None:
        _NC = build()
    return _NC


def _img_T(mat):
    """[R, C] fp32 (R = k*128) -> SBUF image [128, k*C] for T-layout tiles."""
    R, C = mat.shape
    k = R // 128
    return np.ascontiguousarray(
        mat.reshape(k, 128, C).transpose(1, 0, 2).reshape(128, k * C))


def _img_N(mat):
    """[T, F] (T = t*128) -> SBUF image [128, t*F] for N-layout tiles."""
    T, F = mat.shape
    t = T // 128
    return np.ascontiguousarray(
        mat.reshape(t, 128, F).transpose(1, 0, 2).reshape(128, t * F))


def _bias_cols(b):
    """[k*128] -> [128, k] per-partition column layout."""
    return np.ascontiguousarray(b.reshape(-1, 128).T)


def _in_maps(x, g_in_w, g_in_b, g_out_w, g_out_b,
             t_in_w, t_in_b, t_out_w, t_out_b,
             fus_w1, fus_b1, fus_w2, fus_b2,
             ffn_w1, ffn_b1, ffn_w2, ffn_b2,
             gn_g, gn_b, fn_g, fn_b):
    x = np.asarray(x, np.float32)
    f32 = lambda a: np.asarray(a, np.float32)
    bf = lambda a: np.asarray(a, np.float32).astype(BF_NP)

    # shared (same on all cores) tensors
    shared = {
        "wgq": bf(_img_T(f32(g_in_w)[0:256].T)),
        "wgk": bf(_img_T(f32(g_in_w)[256:512].T)),
        "wgv": bf(_img_T(f32(g_in_w)[512:768].T)),
        "wtqk": bf(_img_T(f32(t_in_w)[0:512].T)),
        "wtv": bf(_img_T(f32(t_in_w)[512:768].T)),
        "wgo": bf(_img_T(f32(g_out_w).T)),
        "wto": bf(_img_T(f32(t_out_w).T)),
        "wf1": bf(_img_T(f32(fus_w1).T)),
        "wf2": bf(_img_T(f32(fus_w2).T)),
        "wn1": bf(_img_T(f32(ffn_w1).T)),
        "wn2": bf(_img_T(f32(ffn_w2).T)),
        "bgq": _bias_cols(f32(g_in_b)[0:256]),
        "bgk": _bias_cols(f32(g_in_b)[256:512]),
        "btqk": _bias_cols(f32(t_in_b)[0:512]),
        "bgo": _bias_cols(f32(g_out_b)),
        "bto": _bias_cols(f32(t_out_b)),
        "bf1": _bias_cols(f32(fus_b1)),
        "nbf1": _bias_cols(-f32(fus_b1)),
        "bn1": _bias_cols(f32(ffn_b1)),
        "nbn1": _bias_cols(-f32(ffn_b1)),
        "bgv128": np.ascontiguousarray(
            np.broadcast_to(f32(g_in_b)[512:768], (P, 256))),
        "btv128": np.ascontiguousarray(
            np.broadcast_to(f32(t_in_b)[512:768], (P, 256))),
        "bf2128": np.ascontiguousarray(np.broadcast_to(f32(fus_b2), (P, 256))),
        "bn2128": np.ascontiguousarray(np.broadcast_to(f32(ffn_b2), (P, 256))),
        "gng128": np.ascontiguousarray(np.broadcast_to(f32(gn_g), (P, 256))),
        "gnb128": np.ascontiguousarray(np.broadcast_to(f32(gn_b), (P, 256))),
        "fng128": np.ascontiguousarray(np.broadcast_to(f32(fn_g), (P, 256))),
        "fnb128": np.ascontiguousarray(np.broadcast_to(f32(fn_b), (P, 256))),
    }
    # band mask: key row j valid for query qq iff qq <= j <= qq+4
    jj = np.arange(P)[:, None]
    qq = np.arange(LB)[None, :]
    shared["bandA"] = ((qq <= jj) & (jj <= qq + 4)).astype(BF_NP)

    in_maps = []
    for c in range(8):
        b, hh = c // 2, c % 2
        t0 = 1024 * hh
        xb = x[b]                                    # [2048, 256]
        xq = np.zeros((XQ + 4, D), np.float32)       # rows = x_q tokens t0-2 ..
        lo, hi = max(0, t0 - 2), min(S, t0 + XQ + 2)
        xq[lo - (t0 - 2):hi - (t0 - 2)] = xb[lo:hi]
        xq = xq[:XQ]                                 # guard: only XQ rows used
        bndF = np.ones((P, 1), np.float32)
        bndL = np.ones((P, 1), np.float32)
        if hh == 0:
            bndF[0:2] = 0.0        # keys at tokens -2, -1
        else:
            bndL[34:36] = 0.0      # block-8 keys x_q rows 1026, 1027 (= S, S+1)
        m = dict(shared)
        m["xkvT"] = bf(_img_T(xb.T))
        m["xqT"] = bf(_img_T(xq.T))
        m["xownN"] = _img_N(xb[t0:t0 + 1024])
        m["bndF"] = bndF.astype(BF_NP)
        m["bndL"] = bndL.astype(BF_NP)
        in_maps.append(m)
    return in_maps


def _assemble(results):
    out = np.zeros((B, S, D), np.float32)
    for c in range(8):
        b, hh = c // 2, c % 2
        img = results[c]["out"]                      # [128, 2048]
        chunk = img.reshape(P, 8, 256).transpose(1, 0, 2).reshape(1024, 256)
        out[b, 1024 * hh:1024 * hh + 1024] = chunk
    return out


def kernel(**inputs):
    in_maps = _in_maps(**inputs)
    nc = _get_nc()
    res = run_bass_kernel_spmd(nc, in_maps, core_ids=list(range(8)))
    return _assemble(res.results)



# revision 21
# speedup vs baseline: 1.2394x; 1.2394x over previous
"""Trainium2 Bass kernel for nn_EnhancedTransformerBlock_51917564674691.

Reference block (B=4, S=2048, D=256):
  x_global = global_mha(x, 8 heads, hd=32)          # dense S x S attention
  x_local  = local_mha(x, 4 heads, hd=64, window=5) # banded attention
  x_fused  = MLP_512(silu) over concat([x_global, x_local])
  x        = LN(x + x_fused); x = LN(x + FFN(x)); return x

Sharding: 8 cores = 4 batches x 2 sequence-halves. Each core computes the
full-batch K/V for global attention (needs all 2048 keys) and produces the
output for its 1024 tokens.

v2 design notes:
- ScalarE (ACT) is the hard bottleneck: softmax exp is 16.8M elements/core
  (64 x [128,2048] EXPs ~ 128us at 1 elem/cycle/lane). Everything else is
  arranged to hide under it via the Tile list-scheduler's gap-filling.
- The attention out-projections are folded into fus_w1 host-side
  (W1g = fus_w1[:, :256] @ g_out_w etc.), and all value/out-proj biases ride
  through softmax (attn weights sum to 1) into a folded b1'.
- reciprocal_approx_fast (18-bit) for every softmax denominator and silu.
- silu(x) = (x+b) * 1/(1+exp(-(x+b))): exp on ACT (same table set as the
  softmax), +1 on GpSimd, recip + final fused mul on DVE.
- LayerNorm: sum and sum-of-squares ride on scalar_tensor_tensor accum_out;
  normalize is one fused (x-mu)*istd op. LN1 gain/bias are folded into
  ffn_w1/ffn_b1 for the gemm path; the residual path applies them on GpSimd.
- Local attention packs scores and AV+den into single PSUM banks, one strided
  exp per block, band masks (with boundary baked in) on GpSimd.
"""
import os
import numpy as np
import ml_dtypes

import concourse.bass as bass
import concourse.tile as tile
from concourse import bacc, mybir
from concourse.bass_utils import run_bass_kernel_spmd
from concourse.masks import make_identity

P = 128
BF = mybir.dt.bfloat16
F32 = mybir.dt.float32
BF_NP = ml_dtypes.bfloat16

B, S, D = 4, 2048, 256
TQ = 1024           # tokens per core
XQ = 1152           # padded x_q length (own tokens + halo, zero padded)
NQT = 2             # global q tiles of 512
NKT = 16            # global key tiles of 128
GSC = 1.0 / np.sqrt(32.0)   # global attention scale
LSC = 0.125                 # local attention scale (1/sqrt(64))
LB = 124            # local block queries
NLB = 9             # local blocks (9*124 = 1116 >= 1024)
EPS = 1e-5

AF = mybir.ActivationFunctionType
TT = mybir.AluOpType

# name -> (shape, np dtype) of per-core DRAM inputs (all SBUF-image [128, F])
INPUT_SPECS = {
    "xkvT": ((P, 2 * 2048), BF_NP),   # x[b].T            (full batch, T-layout)
    "wgk": ((P, 2 * 256), BF_NP),
    "bgk": ((P, 2), np.float32),
    "wgq": ((P, 2 * 256), BF_NP),
    "bgq": ((P, 2), np.float32),
    "xqT": ((P, 2 * XQ), BF_NP),      # x_q.T padded      (own + halo, T-layout)
    "wgv": ((P, 2 * 256), BF_NP),
    "wtqk": ((P, 2 * 512), BF_NP),
    "btqk": ((P, 4), np.float32),
    "wtv": ((P, 2 * 256), BF_NP),
    "w1g": ((P, 2 * 512), BF_NP),     # fus_w1[:, :256] @ g_out_w  (T-image)
    "w1t": ((P, 2 * 512), BF_NP),     # fus_w1[:, 256:] @ t_out_w
    "wf2": ((P, 4 * 256), BF_NP),
    "wn1": ((P, 2 * 512), BF_NP),     # ffn_w1 * gn_g (gain folded)
    "wn2": ((P, 4 * 256), BF_NP),
    "bf1": ((P, 4), np.float32),      # folded b1'
    "nbf1": ((P, 4), np.float32),     # -b1'
    "bn1": ((P, 4), np.float32),      # folded bn1'
    "nbn1": ((P, 4), np.float32),
    "resN": ((P, 8 * 256), np.float32),   # x own tokens + fus_b2 (N-image)
    "g128": ((P, 256), np.float32),   # gn_g broadcast
    "b128": ((P, 256), np.float32),   # gn_b + ffn_b2 broadcast
    "fng128": ((P, 256), np.float32),
    "fnb128": ((P, 256), np.float32),
    "bandF": ((P, LB), BF_NP),        # band mask, first block (boundary baked)
    "bandM": ((P, LB), BF_NP),        # band mask, middle blocks
    "bandL": ((P, LB), BF_NP),        # band mask, last block
}


def _patch_act_tables():
    """Make Exp and Ln resolve to the combined natural_log_exp_and_others set
    so the table-load pass emits ONE load instead of thrashing between
    exp_and_others and natural_log."""
    import concourse.hw_specs as hs
    if getattr(hs, "_act_tables_patched", False):
        return
    orig = hs.get_activation_tables

    def patched(module_arch):
        t = dict(orig(module_arch))
        exp = mybir.ActivationFunctionType.Exp
        ln = mybir.ActivationFunctionType.Ln
        for name in ("exp_and_others", "exp_and_friends"):
            if name in t:
                t[name] = t[name] - {exp}
        if "natural_log" in t:
            t["natural_log"] = t["natural_log"] - {ln}
        return t

    hs.get_activation_tables = patched
    import concourse.bacc as bc
    bc.get_activation_tables = patched
    hs._act_tables_patched = True


def build():
    _patch_act_tables()
    nc = bacc.Bacc("TRN2", target_bir_lowering=False, debug=False, num_devices=8)
    dram = {}
    for name, (shape, npdt) in INPUT_SPECS.items():
        dram[name] = nc.dram_tensor(
            name, list(shape), mybir.dt.from_np(np.dtype(npdt)), kind="ExternalInput"
        ).ap()
    out_dram = nc.dram_tensor("out", [P, 8 * 256], F32, kind="ExternalOutput").ap()

    with tile.TileContext(nc) as tc:
        _emit(nc, tc, dram, out_dram)
    nc.compile()
    return nc


def _emit(nc, tc, dram, out_dram):
    from contextlib import ExitStack
    ctx = ExitStack()

    cpool = ctx.enter_context(tc.tile_pool(name="const", bufs=1))
    wpool = ctx.enter_context(tc.tile_pool(name="work", bufs=1))
    spool = ctx.enter_context(tc.tile_pool(name="scratch", bufs=4))
    epool = ctx.enter_context(tc.tile_pool(name="exps", bufs=2))
    pp = ctx.enter_context(tc.tile_pool(name="ps", bufs=1, space="PSUM"))

    def _kernel_body():
        # ---- load constants / inputs --------------------------------------
        cin = {}
        for name, (shape, npdt) in INPUT_SPECS.items():
            t = cpool.tile(list(shape), mybir.dt.from_np(np.dtype(npdt)), tag=name)
            nc.sync.dma_start(t[:], dram[name])
            cin[name] = t

        ones_bf = cpool.tile([P, 64], BF, tag="ones_bf")
        nc.vector.memset(ones_bf[:], 1.0)
        eps_t = cpool.tile([P, 1], F32, tag="eps_t")
        nc.vector.memset(eps_t[:], EPS)
        ident = cpool.tile([P, P], F32, tag="ident")
        make_identity(nc, ident[:])

        # reshaped views of inputs
        xkvT = cin["xkvT"][:].rearrange("p (k n) -> p k n", k=2)     # [128,2,2048]
        xqT = cin["xqT"][:].rearrange("p (k n) -> p k n", k=2)       # [128,2,1152]
        resN = cin["resN"][:].rearrange("p (t f) -> p t f", t=8)     # [128,8,256]
        w = {k: cin[k][:].rearrange("p (k2 n) -> p k2 n", k2=2)
             for k in ("wgq", "wgk", "wgv", "wtqk", "wtv", "w1g", "w1t", "wn1")}
        w["wf2"] = cin["wf2"][:].rearrange("p (k2 n) -> p k2 n", k2=4)
        w["wn2"] = cin["wn2"][:].rearrange("p (k2 n) -> p k2 n", k2=4)

        # ---- persistent intermediates ------------------------------------
        qT = wpool.tile([P, 2, 1024], BF, tag="qT")
        kT = wpool.tile([P, 2, 2048], BF, tag="kT")
        v_aug = wpool.tile([P, NKT, 8, 64], BF, tag="v_aug")
        qkL = wpool.tile([P, 4, XQ], BF, tag="qkL")
        vL = wpool.tile([P, NLB, 256], BF, tag="vL")
        g_oT = wpool.tile([P, 2, 1024], BF, tag="g_oT")
        l_oT = wpool.tile([P, 2, 1024], BF, tag="l_oT")
        h1s = wpool.tile([P, 4, 1024], BF, tag="h1s")
        x1N = wpool.tile([P, 8, 256], F32, tag="x1N")
        res2 = wpool.tile([P, 8, 256], F32, tag="res2")
        x1T = wpool.tile([P, 2, 1024], BF, tag="x1T")
        h2s = wpool.tile([P, 4, 1024], BF, tag="h2s")
        out_sb = wpool.tile([P, 8, 256], F32, tag="out_sb")

        # ones columns of v_aug (denominator trick); GpSimd, it's idle
        nc.gpsimd.memset(v_aug[:, :, :, 32:64], 1.0)

        def ps_sc():
            return pp.tile([P, 2048], F32, tag="sc", name="ps_sc")

        def ps_av():
            return pp.tile([P, 512], F32, tag="av", bufs=2, name="ps_av")

        def ps_sm():
            return pp.tile([P, 512], F32, tag="sm", bufs=2, name="ps_sm")

        # ---- qkv projections (global) ------------------------------------
        # interleave kT/qT so the first global scores are ready ASAP
        def kT_tile(m, nt):
            pm = ps_sm()
            for k in range(2):
                nc.tensor.matmul(pm[:], w["wgk"][:, k, 128 * m:128 * m + 128],
                                 xkvT[:, k, 512 * nt:512 * nt + 512],
                                 start=(k == 0), stop=(k == 1))
            nc.scalar.activation(kT[:, m, 512 * nt:512 * nt + 512], pm[:],
                                 AF.Identity, bias=cin["bgk"][:, m:m + 1])

        def qT_tile(m, nt):
            pm = ps_sm()
            for k in range(2):
                nc.tensor.matmul(pm[:], w["wgq"][:, k, 128 * m:128 * m + 128],
                                 xqT[:, k, 2 + 512 * nt:2 + 512 * nt + 512],
                                 start=(k == 0), stop=(k == 1))
            nc.scalar.activation(qT[:, m, 512 * nt:512 * nt + 512], pm[:],
                                 AF.Identity, bias=cin["bgq"][:, m:m + 1])

        kT_tile(0, 0)
        qT_tile(0, 0)
        kT_tile(0, 1)
        qT_tile(0, 1)
        kT_tile(0, 2)
        kT_tile(0, 3)
        for nt in range(4):
            kT_tile(1, nt)
        for nt in range(2):
            qT_tile(1, nt)

        # v (N-layout, augmented with ones columns): v[key, f] over full batch
        for mt in range(16):
            pm = ps_sm()
            for k in range(2):
                nc.tensor.matmul(pm[:, 0:256], xkvT[:, k, 128 * mt:128 * mt + 128],
                                 w["wgv"][:, k, :], start=(k == 0), stop=(k == 1))
            nc.vector.tensor_copy(
                v_aug[:, mt, :, 0:32],
                pm[:, 0:256].rearrange("p (h d) -> p h d", h=8))

        # ---- qkv projections (local) -------------------------------------
        for m in range(4):
            for nt in range(3):
                pm = ps_sm()
                for k in range(2):
                    nc.tensor.matmul(pm[:, 0:384], w["wtqk"][:, k, 128 * m:128 * m + 128],
                                     xqT[:, k, 384 * nt:384 * nt + 384],
                                     start=(k == 0), stop=(k == 1))
                nc.vector.tensor_tensor(
                    qkL[:, m, 384 * nt:384 * nt + 384], pm[:, 0:384],
                    cin["btqk"][:, m:m + 1].to_broadcast([P, 384]), TT.add)
        for blk in range(NLB):
            pm = ps_sm()
            for k in range(2):
                nc.tensor.matmul(pm[:, 0:256], xqT[:, k, 124 * blk:124 * blk + 128],
                                 w["wtv"][:, k, :], start=(k == 0), stop=(k == 1))
            nc.vector.tensor_copy(vL[:, blk, :], pm[:, 0:256])

        PHASE = int(os.environ.get("KPHASE", "9"))
        if PHASE != 9:
            nc.vector.memset(out_sb[:], 0.0)
            nc.vector.memset(l_oT[:], 0.0)
            nc.vector.memset(g_oT[:], 0.0)
        if PHASE < 2:
            nc.sync.dma_start(out_dram[:, 0:2048],
                              out_sb[:, 0:8, :].rearrange("p t f -> p (t f)"))
            return

        # ---- local attention ---------------------------------------------
        # per block: scores into 4 banks (psum matmul outputs must be
        # bank-aligned), ONE strided exp, one band-mask mult (boundary baked
        # into the per-block mask), AV+den per head-pair, recip + 2 muls.
        for blk in range(NLB):
            k0 = 124 * blk
            q0 = 2 + 124 * blk
            qn = 32 if blk == NLB - 1 else LB  # valid queries in this block
            band = cin["bandF"] if blk == 0 else (
                cin["bandL"] if blk == NLB - 1 else cin["bandM"])
            psc = ps_sc().rearrange("p (l c) -> p l c", l=4)
            for l in range(4):
                r = l % 2
                pt = l // 2
                nc.tensor.matmul(psc[:, l, 0:LB],
                                 qkL[64 * r:64 * r + 64, 2 + pt, k0:k0 + 128],
                                 qkL[64 * r:64 * r + 64, pt, q0:q0 + LB],
                                 start=True, stop=True, tile_position=(64 * r, 0))
            eloc = epool.tile([P, 4, LB], BF, tag="eloc")
            nc.scalar.activation(eloc[:], psc[:, :, 0:LB], AF.Exp, scale=LSC)
            nc.gpsimd.tensor_tensor(eloc[:], eloc[:],
                                    band[:, None, :].to_broadcast([P, 4, LB]),
                                    TT.mult)
            pav = [ps_av(), ps_av()]
            pde = [ps_sm(), ps_sm()]
            for l in range(4):
                pr, c = l // 2, l % 2
                nc.tensor.matmul(pav[pr][64 * c:64 * c + 64, 0:LB],
                                 vL[:, blk, 64 * l:64 * l + 64], eloc[:, l, :],
                                 start=True, stop=True, tile_position=(0, 64 * c))
                nc.tensor.matmul(pde[pr][64 * c:64 * c + 64, 0:LB],
                                 ones_bf[:], eloc[:, l, :],
                                 start=True, stop=True, tile_position=(0, 64 * c))
            for pr in range(2):
                rec = spool.tile([P, LB], F32, tag="lrec")
                nc.vector.reciprocal_approx_fast(rec[:], pde[pr][:, 0:LB])
                nc.vector.tensor_tensor(l_oT[:, pr, k0:k0 + qn],
                                        pav[pr][:, 0:qn],
                                        rec[:, 0:qn], TT.mult)

        if PHASE < 3:
            nc.sync.dma_start(out_dram[:, 0:2048],
                              out_sb[:, 0:8, :].rearrange("p t f -> p (t f)"))
            return

        # ---- global attention --------------------------------------------
        for qt in range(NQT):
            for hg in range(2):
                pav = [ps_av(), ps_av()]
                for kt in range(NKT):
                    psc = ps_sc()
                    for hc in range(4):
                        nc.tensor.matmul(
                            psc[:, 512 * hc:512 * hc + 512],
                            kT[32 * hc:32 * hc + 32, hg, 128 * kt:128 * kt + 128],
                            qT[32 * hc:32 * hc + 32, hg, 512 * qt:512 * qt + 512],
                            start=True, stop=True, tile_position=(32 * hc, 0))
                    eg = epool.tile([P, 2048], BF, tag="eg")
                    nc.scalar.activation(eg[:], psc[:], AF.Exp, scale=GSC)
                    # pair p covers heads 4*hg+2p, 4*hg+2p+1:
                    #   psum rows 0:32 = o(head 2p), 32:64 = den(2p) replicated,
                    #   rows 64:96 = o(2p+1), 96:128 = den(2p+1)
                    # skip_group_check: CoreSim's zero-region tracker is
                    # partition-blind (any two concurrent groups per bank
                    # conflict); HW has per-element has_written bits and the
                    # 64-offset dual-group pattern is verified exact on HW.
                    for p2 in range(2):
                        for c in range(2):
                            h = 4 * hg + 2 * p2 + c
                            nc.tensor.matmul(pav[p2][64 * c:64 * c + 64, :],
                                             v_aug[:, kt, h, :],
                                             eg[:, 512 * (2 * p2 + c):512 * (2 * p2 + c) + 512],
                                             start=(kt == 0), stop=(kt == NKT - 1),
                                             tile_position=(0, 64 * c),
                                             skip_group_check=True)
                qsl = slice(512 * qt, 512 * qt + 512)
                for p2 in range(2):
                    rec = spool.tile([P, 512], F32, tag="grec")
                    # recip of the whole bank; o-rows produce garbage that is
                    # never read (only den rows 32:64 / 96:128 are consumed)
                    nc.vector.reciprocal_approx_fast(rec[:], pav[p2][:])
                    nc.vector.tensor_tensor(g_oT[64 * p2:64 * p2 + 32, hg, qsl],
                                            pav[p2][0:32, :], rec[32:64, :], TT.mult)
                    nc.vector.tensor_tensor(g_oT[64 * p2 + 32:64 * p2 + 64, hg, qsl],
                                            pav[p2][64:96, :], rec[96:128, :], TT.mult)

        if PHASE < 4:
            nc.sync.dma_start(out_dram[:, 0:2048],
                              out_sb[:, 0:8, :].rearrange("p t f -> p (t f)"))
            return

        # ---- MLP tail per chunk ------------------------------------------
        def silu(dst_ap, pm, b_t, nb_t, m):
            """dst = (pm + b) / (1 + exp(-(pm + b))); b is bias column m."""
            e = epool.tile([P, 512], F32, tag="se", name="se")
            nc.scalar.activation(e[:], pm[:], AF.Exp, bias=nb_t[:, m:m + 1],
                                 scale=-1.0)
            t = epool.tile([P, 512], F32, tag="st", name="st")
            nc.gpsimd.tensor_scalar_add(t[:], e[:], 1.0)
            r = epool.tile([P, 512], F32, tag="sr", name="sr")
            nc.vector.reciprocal_approx_fast(r[:], t[:])
            nc.vector.scalar_tensor_tensor(dst_ap, pm[:], b_t[:, m:m + 1], r[:],
                                           TT.add, TT.mult)

        def g2_res_ln(qt, hsrc, w2, res_src, dest, xn_cb):
            """gemm2 (contract 512 -> 256, N-layout out) + residual + LN core.

            dest[:, 4qt+tt, :] = (xr - mu) * istd  where xr = gemm2 + res.
            xn_cb(ta, xn_ap) post-processes the normalized tile.
            """
            xrs = []
            mu_raw = spool.tile([P, 4], F32, tag="mu_raw")
            s2_raw = spool.tile([P, 4], F32, tag="s2_raw")
            for tt in range(4):
                ta = 4 * qt + tt
                pm = ps_sm()
                for k in range(4):
                    nc.tensor.matmul(pm[:, 0:256], hsrc[:, k, 128 * ta:128 * ta + 128],
                                     w2[:, k, :], start=(k == 0), stop=(k == 3))
                xr = spool.tile([P, 256], F32, tag=f"xr{tt}", name="xr")
                nc.vector.scalar_tensor_tensor(
                    xr[:], pm[:, 0:256], 0.0, res_src(ta), TT.add, TT.add,
                    accum_out=mu_raw[:, tt:tt + 1])
                sq = spool.tile([P, 256], F32, tag="sq", name="sq")
                nc.vector.scalar_tensor_tensor(
                    sq[:], xr[:], 1.0, xr[:], TT.mult, TT.mult,
                    accum_out=s2_raw[:, tt:tt + 1])
                xrs.append(xr)
            mu = spool.tile([P, 4], F32, tag="mu")
            mu2 = spool.tile([P, 4], F32, tag="mu2")
            var = spool.tile([P, 4], F32, tag="var")
            istd = spool.tile([P, 4], F32, tag="istd")
            nc.vector.tensor_scalar_mul(mu[:], mu_raw[:], 1.0 / 256.0)
            nc.vector.tensor_tensor(mu2[:], mu[:], mu[:], TT.mult)
            nc.vector.scalar_tensor_tensor(var[:], s2_raw[:], 1.0 / 256.0, mu2[:],
                                           TT.mult, TT.subtract)
            nc.scalar.activation(var[:], var[:], AF.Ln, bias=eps_t[:, 0:1])
            nc.scalar.activation(istd[:], var[:], AF.Exp, scale=-0.5)
            for tt in range(4):
                ta = 4 * qt + tt
                nc.vector.scalar_tensor_tensor(
                    dest[:, ta, :], xrs[tt][:], mu[:, tt:tt + 1],
                    istd[:, tt:tt + 1].to_broadcast([P, 256]),
                    TT.subtract, TT.mult)
                xn_cb(ta, dest[:, ta, :])

        def mlp(qt):
            qsl = slice(512 * qt, 512 * qt + 512)
            # fused MLP gemm1 (out-projections folded in) + silu
            for m in range(4):
                pm = ps_sm()
                nc.tensor.matmul(pm[:], w["w1g"][:, 0, 128 * m:128 * m + 128],
                                 g_oT[:, 0, qsl], start=True, stop=False)
                nc.tensor.matmul(pm[:], w["w1g"][:, 1, 128 * m:128 * m + 128],
                                 g_oT[:, 1, qsl], start=False, stop=False)
                nc.tensor.matmul(pm[:], w["w1t"][:, 0, 128 * m:128 * m + 128],
                                 l_oT[:, 0, qsl], start=False, stop=False)
                nc.tensor.matmul(pm[:], w["w1t"][:, 1, 128 * m:128 * m + 128],
                                 l_oT[:, 1, qsl], start=False, stop=True)
                silu(h1s[:, m, qsl], pm, cin["bf1"], cin["nbf1"], m)

            # gemm2 + residual + LN1 -> x1N (core), res2 = x1N*g + b (gpsimd)
            def ln1_post(ta, xn_ap):
                nc.gpsimd.tensor_tensor(res2[:, ta, :], xn_ap, cin["g128"][:],
                                        TT.mult)
                nc.gpsimd.tensor_tensor(res2[:, ta, :], res2[:, ta, :],
                                        cin["b128"][:], TT.add)
                # transpose x1 chunk -> x1T for the FFN gemm (LN1 gain folded
                # into wn1 host-side, so transpose the core directly)
                for fh in range(2):
                    ptr = ps_sm()
                    nc.tensor.transpose(ptr[:, 0:128],
                                        x1N[:, ta, 128 * fh:128 * fh + 128],
                                        ident[:])
                    nc.vector.tensor_copy(x1T[:, fh, 128 * ta:128 * ta + 128],
                                          ptr[:, 0:128])

            g2_res_ln(qt, h1s, w["wf2"], lambda ta: resN[:, ta, :], x1N, ln1_post)

            # FFN gemm1 + silu
            for m in range(4):
                pm = ps_sm()
                for k in range(2):
                    nc.tensor.matmul(pm[:], w["wn1"][:, k, 128 * m:128 * m + 128],
                                     x1T[:, k, qsl], start=(k == 0), stop=(k == 1))
                silu(h2s[:, m, qsl], pm, cin["bn1"], cin["nbn1"], m)

            # FFN gemm2 + residual(res2) + LN2 -> out_sb (with fn gain/bias)
            def ln2_post(ta, xn_ap):
                nc.gpsimd.tensor_tensor(xn_ap, xn_ap, cin["fng128"][:], TT.mult)
                nc.gpsimd.tensor_tensor(xn_ap, xn_ap, cin["fnb128"][:], TT.add)

            g2_res_ln(qt, h2s, w["wn2"], lambda ta: res2[:, ta, :], out_sb,
                      ln2_post)

            nc.sync.dma_start(out_dram[:, 1024 * qt:1024 * qt + 1024],
                              out_sb[:, 4 * qt:4 * qt + 4, :].rearrange("p t f -> p (t f)"))

        mlp(0)
        mlp(1)

    REPEAT = int(os.environ.get("KREPEAT", "1"))
    if REPEAT > 1:
        with tc.For_i(0, REPEAT, 1):
            _kernel_body()
    else:
        _kernel_body()
    ctx.close()


# ======================================================================
# Host side
# ======================================================================

_NC = None


def _get_nc():
    global _NC
    if _NC is None:
        _NC = build()
    return _NC


def _img_T(mat):
    """[R, C] fp32 (R = k*128) -> SBUF image [128, k*C] for T-layout tiles."""
    R, C = mat.shape
    k = R // 128
    return np.ascontiguousarray(
        mat.reshape(k, 128, C).transpose(1, 0, 2).reshape(128, k * C))


def _img_N(mat):
    """[T, F] (T = t*128) -> SBUF image [128, t*F] for N-layout tiles."""
    T, F = mat.shape
    t = T // 128
    return np.ascontiguousarray(
        mat.reshape(t, 128, F).transpose(1, 0, 2).reshape(128, t * F))


def _bias_cols(b):
    """[k*128] -> [128, k] per-partition column layout."""
    return np.ascontiguousarray(b.reshape(-1, 128).T)


def _in_maps(x, g_in_w, g_in_b, g_out_w, g_out_b,
             t_in_w, t_in_b, t_out_w, t_out_b,
             fus_w1, fus_b1, fus_w2, fus_b2,
             ffn_w1, ffn_b1, ffn_w2, ffn_b2,
             gn_g, gn_b, fn_g, fn_b):
    x = np.asarray(x, np.float32)
    f32 = lambda a: np.asarray(a, np.float32)
    bf = lambda a: np.asarray(a, np.float32).astype(BF_NP)

    g_in_w, g_in_b = f32(g_in_w), f32(g_in_b)
    t_in_w, t_in_b = f32(t_in_w), f32(t_in_b)
    g_out_w, g_out_b = f32(g_out_w), f32(g_out_b)
    t_out_w, t_out_b = f32(t_out_w), f32(t_out_b)
    fus_w1, fus_b1 = f32(fus_w1), f32(fus_b1)
    fus_w2, fus_b2 = f32(fus_w2), f32(fus_b2)
    ffn_w1, ffn_b1 = f32(ffn_w1), f32(ffn_b1)
    ffn_w2, ffn_b2 = f32(ffn_w2), f32(ffn_b2)
    gn_g, gn_b = f32(gn_g), f32(gn_b)
    fn_g, fn_b = f32(fn_g), f32(fn_b)

    # fold out-projections into fus_w1; value/out biases ride through softmax
    W1g = fus_w1[:, 0:256] @ g_out_w            # [512, 256]
    W1t = fus_w1[:, 256:512] @ t_out_w
    b1p = (fus_b1
           + fus_w1[:, 0:256] @ (g_out_w @ g_in_b[512:768] + g_out_b)
           + fus_w1[:, 256:512] @ (t_out_w @ t_in_b[512:768] + t_out_b))
    # fold LN1 gain/bias into FFN gemm1
    wn1p = ffn_w1 * gn_g[None, :]
    bn1p = ffn_b1 + ffn_w1 @ gn_b

    # shared (same on all cores) tensors
    shared = {
        "wgq": bf(_img_T(g_in_w[0:256].T)),
        "wgk": bf(_img_T(g_in_w[256:512].T)),
        "wgv": bf(_img_T(g_in_w[512:768].T)),
        "wtqk": bf(_img_T(t_in_w[0:512].T)),
        "wtv": bf(_img_T(t_in_w[512:768].T)),
        "w1g": bf(_img_T(W1g.T)),
        "w1t": bf(_img_T(W1t.T)),
        "wf2": bf(_img_T(fus_w2.T)),
        "wn1": bf(_img_T(wn1p.T)),
        "wn2": bf(_img_T(ffn_w2.T)),
        "bgq": _bias_cols(g_in_b[0:256]),
        "bgk": _bias_cols(g_in_b[256:512]),
        "btqk": _bias_cols(t_in_b[0:512]),
        "bf1": _bias_cols(b1p),
        "nbf1": _bias_cols(-b1p),
        "bn1": _bias_cols(bn1p),
        "nbn1": _bias_cols(-bn1p),
        "g128": np.ascontiguousarray(np.broadcast_to(gn_g, (P, 256))),
        "b128": np.ascontiguousarray(np.broadcast_to(gn_b + ffn_b2, (P, 256))),
        "fng128": np.ascontiguousarray(np.broadcast_to(fn_g, (P, 256))),
        "fnb128": np.ascontiguousarray(np.broadcast_to(fn_b, (P, 256))),
    }
    # band mask: key row j valid for query qq iff qq <= j <= qq+4
    jj = np.arange(P)[:, None]
    qq = np.arange(LB)[None, :]
    bandA = ((qq <= jj) & (jj <= qq + 4)).astype(np.float32)

    in_maps = []
    for c in range(8):
        b, hh = c // 2, c % 2
        t0 = 1024 * hh
        xb = x[b]                                    # [2048, 256]
        xq = np.zeros((XQ + 4, D), np.float32)       # rows = x_q tokens t0-2 ..
        lo, hi = max(0, t0 - 2), min(S, t0 + XQ + 2)
        xq[lo - (t0 - 2):hi - (t0 - 2)] = xb[lo:hi]
        xq = xq[:XQ]                                 # guard: only XQ rows used
        bandF = bandA.copy()
        bandL = bandA.copy()
        if hh == 0:
            bandF[0:2] = 0.0        # keys at tokens -2, -1
        else:
            bandL[34:36] = 0.0      # block-8 keys x_q rows 1026, 1027 (= S, S+1)
        m = dict(shared)
        m["xkvT"] = bf(_img_T(xb.T))
        m["xqT"] = bf(_img_T(xq.T))
        m["resN"] = _img_N(xb[t0:t0 + 1024] + fus_b2[None, :])
        m["bandF"] = bandF.astype(BF_NP)
        m["bandM"] = bandA.astype(BF_NP)
        m["bandL"] = bandL.astype(BF_NP)
        in_maps.append(m)
    return in_maps


def _assemble(results):
    out = np.zeros((B, S, D), np.float32)
    for c in range(8):
        b, hh = c // 2, c % 2
        img = results[c]["out"]                      # [128, 2048]
        chunk = img.reshape(P, 8, 256).transpose(1, 0, 2).reshape(1024, 256)
        out[b, 1024 * hh:1024 * hh + 1024] = chunk
    return out


def kernel(**inputs):
    in_maps = _in_maps(**inputs)
    nc = _get_nc()
    res = run_bass_kernel_spmd(nc, in_maps, core_ids=list(range(8)))
    return _assemble(res.results)


# revision 26
# speedup vs baseline: 2.0756x; 1.6747x over previous
"""Trainium2 Bass kernel for nn_EnhancedTransformerBlock_51917564674691.

Reference block (B=4, S=2048, D=256):
  x_global = global_mha(x, 8 heads, hd=32)          # dense S x S attention
  x_local  = local_mha(x, 4 heads, hd=64, window=5) # banded attention
  x_fused  = MLP_512(silu) over concat([x_global, x_local])
  x        = LN(x + x_fused); x = LN(x + FFN(x)); return x

Sharding: 8 cores = 4 batches x 2 sequence-halves. Each core computes the
full-batch K/V for global attention (needs all 2048 keys) and produces the
output for its 1024 tokens.

v2 design notes:
- ScalarE (ACT) is the hard bottleneck: softmax exp is 16.8M elements/core
  (64 x [128,2048] EXPs ~ 128us at 1 elem/cycle/lane). Everything else is
  arranged to hide under it via the Tile list-scheduler's gap-filling.
- The attention out-projections are folded into fus_w1 host-side
  (W1g = fus_w1[:, :256] @ g_out_w etc.), and all value/out-proj biases ride
  through softmax (attn weights sum to 1) into a folded b1'.
- reciprocal_approx_fast (18-bit) for every softmax denominator and silu.
- silu(x) = (x+b) * 1/(1+exp(-(x+b))): exp on ACT (same table set as the
  softmax), +1 on GpSimd, recip + final fused mul on DVE.
- LayerNorm: sum and sum-of-squares ride on scalar_tensor_tensor accum_out;
  normalize is one fused (x-mu)*istd op. LN1 gain/bias are folded into
  ffn_w1/ffn_b1 for the gemm path; the residual path applies them on GpSimd.
- Local attention packs scores and AV+den into single PSUM banks, one strided
  exp per block, band masks (with boundary baked in) on GpSimd.
"""
import os
import numpy as np
import ml_dtypes

import concourse.bass as bass
import concourse.tile as tile
from concourse import bacc, mybir
from concourse.bass_utils import run_bass_kernel_spmd
from concourse.masks import make_identity

P = 128
BF = mybir.dt.bfloat16
F32 = mybir.dt.float32
BF_NP = ml_dtypes.bfloat16

B, S, D = 4, 2048, 256
TQ = 1024           # tokens per core
XQ = 1152           # padded x_q length (own tokens + halo, zero padded)
NQT = 2             # global q tiles of 512
NKT = 16            # global key tiles of 128
GSC = 1.0 / np.sqrt(32.0)   # global attention scale
LSC = 0.125                 # local attention scale (1/sqrt(64))
LB = 124            # local block queries
NLB = 9             # local blocks (9*124 = 1116 >= 1024)
EPS = 1e-5

AF = mybir.ActivationFunctionType
TT = mybir.AluOpType

# name -> (shape, np dtype) of per-core DRAM inputs (all SBUF-image [128, F])
INPUT_SPECS = {
    "xkvT": ((P, 2 * 2048), BF_NP),   # x[b].T            (full batch, T-layout)
    "wgk": ((P, 2 * 256), BF_NP),
    "bgk": ((P, 2), np.float32),
    "wgq": ((P, 2 * 256), BF_NP),
    "bgq": ((P, 2), np.float32),
    "xqT": ((P, 2 * XQ), BF_NP),      # x_q.T padded      (own + halo, T-layout)
    "wgv": ((P, 2 * 256), BF_NP),
    "wtqk": ((P, 2 * 512), BF_NP),
    "btqk": ((P, 4), np.float32),
    "wtv": ((P, 2 * 256), BF_NP),
    "w1g": ((P, 2 * 512), BF_NP),     # fus_w1[:, :256] @ g_out_w  (T-image)
    "w1t": ((P, 2 * 512), BF_NP),     # fus_w1[:, 256:] @ t_out_w
    "wf2": ((P, 4 * 256), BF_NP),
    "wn1": ((P, 2 * 512), BF_NP),     # ffn_w1 * gn_g (gain folded)
    "wn2": ((P, 4 * 256), BF_NP),
    "bf1": ((P, 4), np.float32),      # folded b1'
    "nbf1": ((P, 4), np.float32),     # -b1'
    "bn1": ((P, 4), np.float32),      # folded bn1'
    "nbn1": ((P, 4), np.float32),
    "resN": ((P, 8 * 256), np.float32),   # x own tokens + fus_b2 (N-image)
    "g128": ((P, 256), np.float32),   # gn_g broadcast
    "b128": ((P, 256), np.float32),   # gn_b + ffn_b2 broadcast
    "fng128": ((P, 256), np.float32),
    "fnb128": ((P, 256), np.float32),
    "bandF": ((P, LB), BF_NP),        # band mask, first block (boundary baked)
    "bandM": ((P, LB), BF_NP),        # band mask, middle blocks
    "bandL": ((P, LB), BF_NP),        # band mask, last block
}


def _patch_act_tables():
    """Make Exp and Ln resolve to the combined natural_log_exp_and_others set
    so the table-load pass emits ONE load instead of thrashing between
    exp_and_others and natural_log."""
    import concourse.hw_specs as hs
    if getattr(hs, "_act_tables_patched", False):
        return
    orig = hs.get_activation_tables

    def patched(module_arch):
        t = dict(orig(module_arch))
        exp = mybir.ActivationFunctionType.Exp
        ln = mybir.ActivationFunctionType.Ln
        for name in ("exp_and_others", "exp_and_friends"):
            if name in t:
                t[name] = t[name] - {exp}
        if "natural_log" in t:
            t["natural_log"] = t["natural_log"] - {ln}
        return t

    hs.get_activation_tables = patched
    import concourse.bacc as bc
    bc.get_activation_tables = patched
    hs._act_tables_patched = True


def build():
    _patch_act_tables()
    nc = bacc.Bacc("TRN2", target_bir_lowering=False, debug=False, num_devices=8)
    dram = {}
    for name, (shape, npdt) in INPUT_SPECS.items():
        dram[name] = nc.dram_tensor(
            name, list(shape), mybir.dt.from_np(np.dtype(npdt)), kind="ExternalInput"
        ).ap()
    out_dram = nc.dram_tensor("out", [P, 8 * 256], F32, kind="ExternalOutput").ap()

    with tile.TileContext(nc) as tc:
        _emit(nc, tc, dram, out_dram)
    nc.compile()
    return nc


def _emit(nc, tc, dram, out_dram):
    from contextlib import ExitStack
    ctx = ExitStack()

    cpool = ctx.enter_context(tc.tile_pool(name="const", bufs=1))
    wpool = ctx.enter_context(tc.tile_pool(name="work", bufs=1))
    spool = ctx.enter_context(tc.tile_pool(name="scratch", bufs=4))
    epool = ctx.enter_context(tc.tile_pool(name="exps", bufs=2))
    pp = ctx.enter_context(tc.tile_pool(name="ps", bufs=1, space="PSUM"))

    def _kernel_body():
        # ---- load constants / inputs --------------------------------------
        cin = {}
        for name, (shape, npdt) in INPUT_SPECS.items():
            t = cpool.tile(list(shape), mybir.dt.from_np(np.dtype(npdt)), tag=name)
            nc.sync.dma_start(t[:], dram[name])
            cin[name] = t

        ones_bf = cpool.tile([P, 64], BF, tag="ones_bf")
        nc.vector.memset(ones_bf[:], 1.0)
        eps_t = cpool.tile([P, 1], F32, tag="eps_t")
        nc.vector.memset(eps_t[:], EPS)
        ident = cpool.tile([P, P], F32, tag="ident")
        make_identity(nc, ident[:])

        # reshaped views of inputs
        xkvT = cin["xkvT"][:].rearrange("p (k n) -> p k n", k=2)     # [128,2,2048]
        xqT = cin["xqT"][:].rearrange("p (k n) -> p k n", k=2)       # [128,2,1152]
        resN = cin["resN"][:].rearrange("p (t f) -> p t f", t=8)     # [128,8,256]
        w = {k: cin[k][:].rearrange("p (k2 n) -> p k2 n", k2=2)
             for k in ("wgq", "wgk", "wgv", "wtqk", "wtv", "w1g", "w1t", "wn1")}
        w["wf2"] = cin["wf2"][:].rearrange("p (k2 n) -> p k2 n", k2=4)
        w["wn2"] = cin["wn2"][:].rearrange("p (k2 n) -> p k2 n", k2=4)

        # ---- persistent intermediates ------------------------------------
        qT = wpool.tile([P, 2, 1024], BF, tag="qT")
        kT = wpool.tile([P, 2, 2048], BF, tag="kT")
        v_aug = wpool.tile([P, NKT, 8, 64], BF, tag="v_aug")
        qkL = wpool.tile([P, 4, XQ], BF, tag="qkL")
        vL = wpool.tile([P, NLB, 256], BF, tag="vL")
        g_oT = wpool.tile([P, 2, 1024], BF, tag="g_oT")
        l_oT = wpool.tile([P, 2, 1024], BF, tag="l_oT")
        h1s = wpool.tile([P, 4, 1024], BF, tag="h1s")
        x1N = wpool.tile([P, 8, 256], F32, tag="x1N")
        res2 = wpool.tile([P, 8, 256], F32, tag="res2")
        x1T = wpool.tile([P, 2, 1024], BF, tag="x1T")
        h2s = wpool.tile([P, 4, 1024], BF, tag="h2s")
        out_sb = wpool.tile([P, 8, 256], F32, tag="out_sb")

        # ones columns of v_aug (denominator trick); GpSimd, it's idle
        nc.vector.memset(v_aug[:, :, :, 32:64], 1.0)

        # two independent half-score tiles so exp of one half overlaps
        # scores/AV of the other (pipelines ACT to ~continuous duty)
        def ps_scA():
            return pp.tile([P, 1024], F32, tag="scA", name="ps_scA")

        def ps_scB():
            return pp.tile([P, 1024], F32, tag="scB", name="ps_scB")

        def ps_av():
            return pp.tile([P, 512], F32, tag="av", bufs=2, name="ps_av")

        def ps_sm():
            return pp.tile([P, 512], F32, tag="sm", bufs=2, name="ps_sm")

        # ---- qkv projections (global) ------------------------------------
        # interleave kT/qT so the first global scores are ready ASAP
        def kT_tile(m, nt):
            pm = ps_sm()
            for k in range(2):
                nc.tensor.matmul(pm[:], w["wgk"][:, k, 128 * m:128 * m + 128],
                                 xkvT[:, k, 512 * nt:512 * nt + 512],
                                 start=(k == 0), stop=(k == 1))
            nc.scalar.activation(kT[:, m, 512 * nt:512 * nt + 512], pm[:],
                                 AF.Identity, bias=cin["bgk"][:, m:m + 1])

        def qT_tile(m, nt):
            pm = ps_sm()
            for k in range(2):
                nc.tensor.matmul(pm[:], w["wgq"][:, k, 128 * m:128 * m + 128],
                                 xqT[:, k, 2 + 512 * nt:2 + 512 * nt + 512],
                                 start=(k == 0), stop=(k == 1))
            nc.scalar.activation(qT[:, m, 512 * nt:512 * nt + 512], pm[:],
                                 AF.Identity, bias=cin["bgq"][:, m:m + 1])

        kT_tile(0, 0)
        qT_tile(0, 0)
        kT_tile(0, 1)
        qT_tile(0, 1)
        kT_tile(0, 2)
        kT_tile(0, 3)
        for nt in range(4):
            kT_tile(1, nt)
        for nt in range(2):
            qT_tile(1, nt)

        # v (N-layout, augmented with ones columns): v[key, f] over full batch
        for mt in range(16):
            pm = ps_sm()
            for k in range(2):
                nc.tensor.matmul(pm[:, 0:256], xkvT[:, k, 128 * mt:128 * mt + 128],
                                 w["wgv"][:, k, :], start=(k == 0), stop=(k == 1))
            nc.vector.tensor_copy(
                v_aug[:, mt, :, 0:32],
                pm[:, 0:256].rearrange("p (h d) -> p h d", h=8))

        # ---- qkv projections (local) -------------------------------------
        for m in range(4):
            for nt in range(3):
                pm = ps_sm()
                for k in range(2):
                    nc.tensor.matmul(pm[:, 0:384], w["wtqk"][:, k, 128 * m:128 * m + 128],
                                     xqT[:, k, 384 * nt:384 * nt + 384],
                                     start=(k == 0), stop=(k == 1))
                nc.vector.tensor_tensor(
                    qkL[:, m, 384 * nt:384 * nt + 384], pm[:, 0:384],
                    cin["btqk"][:, m:m + 1].to_broadcast([P, 384]), TT.add)
        for blk in range(NLB):
            pm = ps_sm()
            for k in range(2):
                nc.tensor.matmul(pm[:, 0:256], xqT[:, k, 124 * blk:124 * blk + 128],
                                 w["wtv"][:, k, :], start=(k == 0), stop=(k == 1))
            nc.vector.tensor_copy(vL[:, blk, :], pm[:, 0:256])

        PHASE = int(os.environ.get("KPHASE", "9"))
        if PHASE != 9:
            nc.vector.memset(out_sb[:], 0.0)
            nc.vector.memset(l_oT[:], 0.0)
            nc.vector.memset(g_oT[:], 0.0)
        if PHASE < 2:
            nc.sync.dma_start(out_dram[:, 0:2048],
                              out_sb[:, 0:8, :].rearrange("p t f -> p (t f)"))
            return

        # ---- local attention ---------------------------------------------
        # per block: scores into 4 banks (psum matmul outputs must be
        # bank-aligned), ONE strided exp, one band-mask mult (boundary baked
        # into the per-block mask), AV+den per head-pair, recip + 2 muls.
        for blk in range(NLB):
            k0 = 124 * blk
            q0 = 2 + 124 * blk
            qn = 32 if blk == NLB - 1 else LB  # valid queries in this block
            band = cin["bandF"] if blk == 0 else (
                cin["bandL"] if blk == NLB - 1 else cin["bandM"])
            pscs = [ps_scA().rearrange("p (l c) -> p l c", l=2),
                    ps_scB().rearrange("p (l c) -> p l c", l=2)]
            for l in range(4):
                r = l % 2
                pt = l // 2
                nc.tensor.matmul(pscs[l // 2][:, l % 2, 0:LB],
                                 qkL[64 * r:64 * r + 64, 2 + pt, k0:k0 + 128],
                                 qkL[64 * r:64 * r + 64, pt, q0:q0 + LB],
                                 start=True, stop=True, tile_position=(64 * r, 0))
            eloc = epool.tile([P, 4, LB], BF, tag="eloc")
            for h2 in range(2):
                nc.scalar.activation(eloc[:, 2 * h2:2 * h2 + 2, :],
                                     pscs[h2][:, :, 0:LB], AF.Exp, scale=LSC)
            nc.vector.tensor_tensor(eloc[:], eloc[:],
                                    band[:, None, :].to_broadcast([P, 4, LB]),
                                    TT.mult)
            pav = [ps_av(), ps_av()]
            pde = [ps_sm(), ps_sm()]
            for l in range(4):
                pr, c = l // 2, l % 2
                nc.tensor.matmul(pav[pr][64 * c:64 * c + 64, 0:LB],
                                 vL[:, blk, 64 * l:64 * l + 64], eloc[:, l, :],
                                 start=True, stop=True, tile_position=(0, 64 * c))
                nc.tensor.matmul(pde[pr][64 * c:64 * c + 64, 0:LB],
                                 ones_bf[:], eloc[:, l, :],
                                 start=True, stop=True, tile_position=(0, 64 * c))
            for pr in range(2):
                rec = spool.tile([P, LB], F32, tag="lrec")
                nc.vector.reciprocal_approx_fast(rec[:], pde[pr][:, 0:LB])
                nc.vector.tensor_tensor(l_oT[:, pr, k0:k0 + qn],
                                        pav[pr][:, 0:qn],
                                        rec[:, 0:qn], TT.mult)

        if PHASE < 3:
            nc.sync.dma_start(out_dram[:, 0:2048],
                              out_sb[:, 0:8, :].rearrange("p t f -> p (t f)"))
            return

        # ---- global attention --------------------------------------------
        # per kt: scores for head-pair A into scA, pair B into scB; exp-A and
        # exp-B are separate ACT instructions, so scores/AV of half A overlap
        # exp of half B and ACT stays ~continuously busy.
        for qt in range(NQT):
            for hg in range(2):
                pav = [ps_av(), ps_av()]
                for kt in range(NKT):
                    pscs = [ps_scA(), ps_scB()]
                    egs = [epool.tile([P, 1024], BF, tag="egA", name="egA"),
                           epool.tile([P, 1024], BF, tag="egB", name="egB")]
                    for p2 in range(2):
                        for c in range(2):
                            hc = 2 * p2 + c
                            nc.tensor.matmul(
                                pscs[p2][:, 512 * c:512 * c + 512],
                                kT[32 * hc:32 * hc + 32, hg, 128 * kt:128 * kt + 128],
                                qT[32 * hc:32 * hc + 32, hg, 512 * qt:512 * qt + 512],
                                start=True, stop=True, tile_position=(32 * hc, 0))
                        nc.scalar.activation(egs[p2][:], pscs[p2][:], AF.Exp,
                                             scale=GSC)
                        # pair p2 covers heads 4*hg+2p2, 4*hg+2p2+1:
                        #   psum rows 0:32 = o(head), 32:64 = den replicated,
                        #   rows 64:96 = o(head+1), 96:128 = den(head+1)
                        # skip_group_check: CoreSim's zero-region tracker is
                        # partition-blind (any two concurrent groups per bank
                        # conflict); HW has per-element has_written bits and
                        # the 64-offset dual-group pattern is exact on HW.
                        for c in range(2):
                            h = 4 * hg + 2 * p2 + c
                            nc.tensor.matmul(pav[p2][64 * c:64 * c + 64, :],
                                             v_aug[:, kt, h, :],
                                             egs[p2][:, 512 * c:512 * c + 512],
                                             start=(kt == 0), stop=(kt == NKT - 1),
                                             tile_position=(0, 64 * c),
                                             skip_group_check=True)
                qsl = slice(512 * qt, 512 * qt + 512)
                for p2 in range(2):
                    rec = spool.tile([P, 512], F32, tag="grec")
                    # recip of the whole bank; o-rows produce garbage that is
                    # never read (only den rows 32:64 / 96:128 are consumed)
                    nc.vector.reciprocal_approx_fast(rec[:], pav[p2][:])
                    nc.vector.tensor_tensor(g_oT[64 * p2:64 * p2 + 32, hg, qsl],
                                            pav[p2][0:32, :], rec[32:64, :], TT.mult)
                    nc.vector.tensor_tensor(g_oT[64 * p2 + 32:64 * p2 + 64, hg, qsl],
                                            pav[p2][64:96, :], rec[96:128, :], TT.mult)

        if PHASE < 4:
            nc.sync.dma_start(out_dram[:, 0:2048],
                              out_sb[:, 0:8, :].rearrange("p t f -> p (t f)"))
            return

        # ---- MLP tail per chunk ------------------------------------------
        def silu(dst_ap, pm, b_t, nb_t, m):
            """dst = (pm + b) / (1 + exp(-(pm + b))); b is bias column m."""
            e = epool.tile([P, 512], F32, tag="se", name="se")
            nc.scalar.activation(e[:], pm[:], AF.Exp, bias=nb_t[:, m:m + 1],
                                 scale=-1.0)
            t = epool.tile([P, 512], F32, tag="st", name="st")
            nc.vector.tensor_scalar_add(t[:], e[:], 1.0)
            r = epool.tile([P, 512], F32, tag="sr", name="sr")
            nc.vector.reciprocal_approx_fast(r[:], t[:])
            nc.vector.scalar_tensor_tensor(dst_ap, pm[:], b_t[:, m:m + 1], r[:],
                                           TT.add, TT.mult)

        def g2_res_ln(qt, hsrc, w2, res_src, dest, xn_cb):
            """gemm2 (contract 512 -> 256, N-layout out) + residual + LN core.

            dest[:, 4qt+tt, :] = (xr - mu) * istd  where xr = gemm2 + res.
            xn_cb(ta, xn_ap) post-processes the normalized tile.
            """
            xrs = []
            mu_raw = spool.tile([P, 4], F32, tag="mu_raw")
            s2_raw = spool.tile([P, 4], F32, tag="s2_raw")
            for tt in range(4):
                ta = 4 * qt + tt
                pm = ps_sm()
                for k in range(4):
                    nc.tensor.matmul(pm[:, 0:256], hsrc[:, k, 128 * ta:128 * ta + 128],
                                     w2[:, k, :], start=(k == 0), stop=(k == 3))
                xr = spool.tile([P, 256], F32, tag=f"xr{tt}", name="xr")
                nc.vector.scalar_tensor_tensor(
                    xr[:], pm[:, 0:256], 0.0, res_src(ta), TT.add, TT.add,
                    accum_out=mu_raw[:, tt:tt + 1])
                sq = spool.tile([P, 256], F32, tag="sq", name="sq")
                nc.vector.scalar_tensor_tensor(
                    sq[:], xr[:], 1.0, xr[:], TT.mult, TT.mult,
                    accum_out=s2_raw[:, tt:tt + 1])
                xrs.append(xr)
            mu = spool.tile([P, 4], F32, tag="mu")
            mu2 = spool.tile([P, 4], F32, tag="mu2")
            var = spool.tile([P, 4], F32, tag="var")
            istd = spool.tile([P, 4], F32, tag="istd")
            nc.vector.tensor_scalar_mul(mu[:], mu_raw[:], 1.0 / 256.0)
            nc.vector.tensor_tensor(mu2[:], mu[:], mu[:], TT.mult)
            nc.vector.scalar_tensor_tensor(var[:], s2_raw[:], 1.0 / 256.0, mu2[:],
                                           TT.mult, TT.subtract)
            nc.scalar.activation(var[:], var[:], AF.Ln, bias=eps_t[:, 0:1])
            nc.scalar.activation(istd[:], var[:], AF.Exp, scale=-0.5)
            for tt in range(4):
                ta = 4 * qt + tt
                nc.vector.scalar_tensor_tensor(
                    dest[:, ta, :], xrs[tt][:], mu[:, tt:tt + 1],
                    istd[:, tt:tt + 1].to_broadcast([P, 256]),
                    TT.subtract, TT.mult)
                xn_cb(ta, dest[:, ta, :])

        def mlp(qt):
            qsl = slice(512 * qt, 512 * qt + 512)
            # fused MLP gemm1 (out-projections folded in) + silu
            for m in range(4):
                pm = ps_sm()
                nc.tensor.matmul(pm[:], w["w1g"][:, 0, 128 * m:128 * m + 128],
                                 g_oT[:, 0, qsl], start=True, stop=False)
                nc.tensor.matmul(pm[:], w["w1g"][:, 1, 128 * m:128 * m + 128],
                                 g_oT[:, 1, qsl], start=False, stop=False)
                nc.tensor.matmul(pm[:], w["w1t"][:, 0, 128 * m:128 * m + 128],
                                 l_oT[:, 0, qsl], start=False, stop=False)
                nc.tensor.matmul(pm[:], w["w1t"][:, 1, 128 * m:128 * m + 128],
                                 l_oT[:, 1, qsl], start=False, stop=True)
                silu(h1s[:, m, qsl], pm, cin["bf1"], cin["nbf1"], m)

            # gemm2 + residual + LN1 -> x1N (core), res2 = x1N*g + b (gpsimd)
            def ln1_post(ta, xn_ap):
                nc.vector.tensor_tensor(res2[:, ta, :], xn_ap, cin["g128"][:],
                                        TT.mult)
                nc.vector.tensor_tensor(res2[:, ta, :], res2[:, ta, :],
                                        cin["b128"][:], TT.add)
                # transpose x1 chunk -> x1T for the FFN gemm (LN1 gain folded
                # into wn1 host-side, so transpose the core directly)
                for fh in range(2):
                    ptr = ps_sm()
                    nc.tensor.transpose(ptr[:, 0:128],
                                        x1N[:, ta, 128 * fh:128 * fh + 128],
                                        ident[:])
                    nc.vector.tensor_copy(x1T[:, fh, 128 * ta:128 * ta + 128],
                                          ptr[:, 0:128])

            g2_res_ln(qt, h1s, w["wf2"], lambda ta: resN[:, ta, :], x1N, ln1_post)

            # FFN gemm1 + silu
            for m in range(4):
                pm = ps_sm()
                for k in range(2):
                    nc.tensor.matmul(pm[:], w["wn1"][:, k, 128 * m:128 * m + 128],
                                     x1T[:, k, qsl], start=(k == 0), stop=(k == 1))
                silu(h2s[:, m, qsl], pm, cin["bn1"], cin["nbn1"], m)

            # FFN gemm2 + residual(res2) + LN2 -> out_sb (with fn gain/bias)
            def ln2_post(ta, xn_ap):
                nc.vector.tensor_tensor(xn_ap, xn_ap, cin["fng128"][:], TT.mult)
                nc.vector.tensor_tensor(xn_ap, xn_ap, cin["fnb128"][:], TT.add)

            g2_res_ln(qt, h2s, w["wn2"], lambda ta: res2[:, ta, :], out_sb,
                      ln2_post)

            nc.sync.dma_start(out_dram[:, 1024 * qt:1024 * qt + 1024],
                              out_sb[:, 4 * qt:4 * qt + 4, :].rearrange("p t f -> p (t f)"))

        mlp(0)
        mlp(1)

    REPEAT = int(os.environ.get("KREPEAT", "1"))
    if REPEAT > 1:
        with tc.For_i(0, REPEAT, 1):
            _kernel_body()
    else:
        _kernel_body()
    ctx.close()


# ======================================================================
# Host side
# ======================================================================

_NC = None


def _get_nc():
    global _NC
    if _NC is None:
        _NC = build()
    return _NC


def _img_T(mat):
    """[R, C] fp32 (R = k*128) -> SBUF image [128, k*C] for T-layout tiles."""
    R, C = mat.shape
    k = R // 128
    return np.ascontiguousarray(
        mat.reshape(k, 128, C).transpose(1, 0, 2).reshape(128, k * C))


def _img_N(mat):
    """[T, F] (T = t*128) -> SBUF image [128, t*F] for N-layout tiles."""
    T, F = mat.shape
    t = T // 128
    return np.ascontiguousarray(
        mat.reshape(t, 128, F).transpose(1, 0, 2).reshape(128, t * F))


def _bias_cols(b):
    """[k*128] -> [128, k] per-partition column layout."""
    return np.ascontiguousarray(b.reshape(-1, 128).T)


def _in_maps(x, g_in_w, g_in_b, g_out_w, g_out_b,
             t_in_w, t_in_b, t_out_w, t_out_b,
             fus_w1, fus_b1, fus_w2, fus_b2,
             ffn_w1, ffn_b1, ffn_w2, ffn_b2,
             gn_g, gn_b, fn_g, fn_b):
    x = np.asarray(x, np.float32)
    f32 = lambda a: np.asarray(a, np.float32)
    bf = lambda a: np.asarray(a, np.float32).astype(BF_NP)

    g_in_w, g_in_b = f32(g_in_w), f32(g_in_b)
    t_in_w, t_in_b = f32(t_in_w), f32(t_in_b)
    g_out_w, g_out_b = f32(g_out_w), f32(g_out_b)
    t_out_w, t_out_b = f32(t_out_w), f32(t_out_b)
    fus_w1, fus_b1 = f32(fus_w1), f32(fus_b1)
    fus_w2, fus_b2 = f32(fus_w2), f32(fus_b2)
    ffn_w1, ffn_b1 = f32(ffn_w1), f32(ffn_b1)
    ffn_w2, ffn_b2 = f32(ffn_w2), f32(ffn_b2)
    gn_g, gn_b = f32(gn_g), f32(gn_b)
    fn_g, fn_b = f32(fn_g), f32(fn_b)

    # fold out-projections into fus_w1; value/out biases ride through softmax
    W1g = fus_w1[:, 0:256] @ g_out_w            # [512, 256]
    W1t = fus_w1[:, 256:512] @ t_out_w
    b1p = (fus_b1
           + fus_w1[:, 0:256] @ (g_out_w @ g_in_b[512:768] + g_out_b)
           + fus_w1[:, 256:512] @ (t_out_w @ t_in_b[512:768] + t_out_b))
    # fold LN1 gain/bias into FFN gemm1
    wn1p = ffn_w1 * gn_g[None, :]
    bn1p = ffn_b1 + ffn_w1 @ gn_b

    # shared (same on all cores) tensors
    shared = {
        "wgq": bf(_img_T(g_in_w[0:256].T)),
        "wgk": bf(_img_T(g_in_w[256:512].T)),
        "wgv": bf(_img_T(g_in_w[512:768].T)),
        "wtqk": bf(_img_T(t_in_w[0:512].T)),
        "wtv": bf(_img_T(t_in_w[512:768].T)),
        "w1g": bf(_img_T(W1g.T)),
        "w1t": bf(_img_T(W1t.T)),
        "wf2": bf(_img_T(fus_w2.T)),
        "wn1": bf(_img_T(wn1p.T)),
        "wn2": bf(_img_T(ffn_w2.T)),
        "bgq": _bias_cols(g_in_b[0:256]),
        "bgk": _bias_cols(g_in_b[256:512]),
        "btqk": _bias_cols(t_in_b[0:512]),
        "bf1": _bias_cols(b1p),
        "nbf1": _bias_cols(-b1p),
        "bn1": _bias_cols(bn1p),
        "nbn1": _bias_cols(-bn1p),
        "g128": np.ascontiguousarray(np.broadcast_to(gn_g, (P, 256))),
        "b128": np.ascontiguousarray(np.broadcast_to(gn_b + ffn_b2, (P, 256))),
        "fng128": np.ascontiguousarray(np.broadcast_to(fn_g, (P, 256))),
        "fnb128": np.ascontiguousarray(np.broadcast_to(fn_b, (P, 256))),
    }
    # band mask: key row j valid for query qq iff qq <= j <= qq+4
    jj = np.arange(P)[:, None]
    qq = np.arange(LB)[None, :]
    bandA = ((qq <= jj) & (jj <= qq + 4)).astype(np.float32)

    in_maps = []
    for c in range(8):
        b, hh = c // 2, c % 2
        t0 = 1024 * hh
        xb = x[b]                                    # [2048, 256]
        xq = np.zeros((XQ + 4, D), np.float32)       # rows = x_q tokens t0-2 ..
        lo, hi = max(0, t0 - 2), min(S, t0 + XQ + 2)
        xq[lo - (t0 - 2):hi - (t0 - 2)] = xb[lo:hi]
        xq = xq[:XQ]                                 # guard: only XQ rows used
        bandF = bandA.copy()
        bandL = bandA.copy()
        if hh == 0:
            bandF[0:2] = 0.0        # keys at tokens -2, -1
        else:
            bandL[34:36] = 0.0      # block-8 keys x_q rows 1026, 1027 (= S, S+1)
        m = dict(shared)
        m["xkvT"] = bf(_img_T(xb.T))
        m["xqT"] = bf(_img_T(xq.T))
        m["resN"] = _img_N(xb[t0:t0 + 1024] + fus_b2[None, :])
        m["bandF"] = bandF.astype(BF_NP)
        m["bandM"] = bandA.astype(BF_NP)
        m["bandL"] = bandL.astype(BF_NP)
        in_maps.append(m)
    return in_maps


def _assemble(results):
    out = np.zeros((B, S, D), np.float32)
    for c in range(8):
        b, hh = c // 2, c % 2
        img = results[c]["out"]                      # [128, 2048]
        chunk = img.reshape(P, 8, 256).transpose(1, 0, 2).reshape(1024, 256)
        out[b, 1024 * hh:1024 * hh + 1024] = chunk
    return out


def kernel(**inputs):
    in_maps = _in_maps(**inputs)
    nc = _get_nc()
    res = run_bass_kernel_spmd(nc, in_maps, core_ids=list(range(8)))
    return _assemble(res.results)
